# revision 4
# baseline (speedup 1.0000x reference)
"""Biaffine kernel for Trainium2 (8 NeuronCores, Bass/Tile).

out[b,x,y,o] = sum_ij X[b,x,i] w1[i,o,j] Y[b,y,j]
             + (X[b] @ w2[:D])[x,o] + (Y[b] @ w2[D:2D])[y,o] + w2[2D][o]

Sharding: tensor-parallel over o (the w1/w2 out_size axis): core c owns
o in [16c, 16c+16). Each core reads the full (transposed) inputs, its w1/w2
slice, and writes out[b, o_local, x, y]; the host reorders to [b,x,y,o].

Per-core schedule, per (o, batch-pair):
  stage1: M[j, x2] = sum_i W_o[i,j] XT[i, x2]        (PE, fp32r, N=512)
  fold:   M += w2b[j,o]      (DVE tensor_scalar_add on PSUM->SBUF copy)
  stage2: out[x, y] = sum_j M[j, x] YT[j, y]         (PE, fp32r, N=256)
  fold:   out += c1b[x, o]   (ACT Identity+bias on PSUM->SBUF copy)
where c1b = X[b] @ w2a + bias is computed once per (b, x-tile) upfront.
"""

import numpy as np

B, L, D, O = 8, 256, 512, 128
NCORES = 8
OS = O // NCORES     # 16 o-channels per core
IC = D // 128        # 4 contraction chunks of 128
NBP = B // 2         # 4 batch pairs (stage1 moving dim = 2*L = 512)

_CACHE = {}


def _build(mmdt_name: str):
    """Build + compile the per-core Bass program (same program on all cores)."""
    import concourse.tile as tile
    from concourse import bacc, mybir

    if mmdt_name in _CACHE:
        return _CACHE[mmdt_name]

    F32 = mybir.dt.float32
    MMDT = getattr(mybir.dt, mmdt_name)

    nc = bacc.Bacc("TRN2", target_bir_lowering=False, debug=False,
                   num_devices=NCORES)

    x1t_d = nc.dram_tensor("x1t", [B, D, L], MMDT, kind="ExternalInput")
    x2t_d = nc.dram_tensor("x2t", [B, D, L], MMDT, kind="ExternalInput")
    w1s_d = nc.dram_tensor("w1s", [OS, IC, 128, D], MMDT, kind="ExternalInput")
    w2a_d = nc.dram_tensor("w2a", [128, IC, OS], MMDT, kind="ExternalInput")
    w2b_d = nc.dram_tensor("w2b", [128, IC, OS], F32, kind="ExternalInput")
    bias_d = nc.dram_tensor("bias", [1, OS], MMDT, kind="ExternalInput")
    ones_d = nc.dram_tensor("ones", [1, 128], MMDT, kind="ExternalInput")
    out_d = nc.dram_tensor("out", [B, OS, L, L], F32, kind="ExternalOutput")


    with tile.TileContext(nc) as tc:
        with tc.tile_pool(name="small", bufs=1) as small, \
             tc.tile_pool(name="xy", bufs=1) as xy, \
             tc.tile_pool(name="wp", bufs=4) as wp, \
             tc.tile_pool(name="mp", bufs=3) as mp, \
             tc.tile_pool(name="op", bufs=8) as op, \
             tc.tile_pool(name="ps1", bufs=2, space="PSUM") as ps1, \
             tc.tile_pool(name="ps2", bufs=4, space="PSUM") as ps2:

            # --- small persistent tiles ---
            w2a_sb = small.tile([128, IC, OS], MMDT, tag="w2a")
            w2b_sb = small.tile([128, IC, OS], F32, tag="w2b")
            bias_sb = small.tile([1, OS], MMDT, tag="bias")
            ones_sb = small.tile([1, 128], MMDT, tag="ones")
            c1b_sb = small.tile([128, B * 2 * OS], F32, tag="c1b")
            nc.sync.dma_start(out=w2a_sb, in_=w2a_d.ap())
            nc.sync.dma_start(out=w2b_sb, in_=w2b_d.ap())
            nc.sync.dma_start(out=bias_sb, in_=bias_d.ap())
            nc.sync.dma_start(out=ones_sb, in_=ones_d.ap())

            # --- transposed inputs, all batches resident: [i%128, ic, b_in, x] ---
            xts, yts = [], []
            for bp in range(NBP):
                xt = xy.tile([128, IC, 2, L], MMDT, tag=f"xt{bp}")
                yt = xy.tile([128, IC, 2, L], MMDT, tag=f"yt{bp}")
                for b_in in range(2):
                    b = 2 * bp + b_in
                    for ic in range(IC):
                        nc.sync.dma_start(
                            out=xt[:, ic, b_in, :],
                            in_=x1t_d.ap()[b, ic * 128:(ic + 1) * 128, :])
                        nc.sync.dma_start(
                            out=yt[:, ic, b_in, :],
                            in_=x2t_d.ap()[b, ic * 128:(ic + 1) * 128, :])
                xts.append(xt)
                yts.append(yt)

            # --- c1b[x, (b,xt,o)] = X[b] @ w2a + bias, all (b, xt) upfront ---
            with tc.tile_pool(name="psc", bufs=2, space="PSUM") as psc:
                for b in range(B):
                    bp, b_in = divmod(b, 2)
                    for xt_i in range(2):
                        pc = psc.tile([128, OS], F32, tag="pc")
                        for ic in range(IC):
                            nc.tensor.matmul(
                                pc,
                                xts[bp][:, ic, b_in, xt_i * 128:(xt_i + 1) * 128],
                                w2a_sb[:, ic, :],
                                start=(ic == 0), stop=False)
                        nc.tensor.matmul(
                            pc, ones_sb[0:1, :], bias_sb[0:1, :],
                            start=False, stop=True)
                        nc.vector.tensor_copy(
                            c1b_sb[:, (b * 2 + xt_i) * OS:(b * 2 + xt_i + 1) * OS],
                            pc)

            # --- main loop: software-pipelined over (o, bp) ---
            def stage2(o, bp, m2):
                for b_in in range(2):
                    b = 2 * bp + b_in
                    for xt_i in range(2):
                        p2 = ps2.tile([128, L], F32, tag="p2")
                        for jc in range(IC):
                            nc.tensor.matmul(
                                p2,
                                m2[:, jc, b_in, xt_i * 128:(xt_i + 1) * 128],
                                yts[bp][:, jc, b_in, :],
                                start=(jc == 0), stop=(jc == IC - 1))
                        o_sb = op.tile([128, L], F32, tag="osb")
                        nc.scalar.add(
                            o_sb, p2,
                            c1b_sb[:, (b * 2 + xt_i) * OS + o:
                                   (b * 2 + xt_i) * OS + o + 1])
                        nc.sync.dma_start(
                            out=out_d.ap()[b, o, xt_i * 128:(xt_i + 1) * 128, :],
                            in_=o_sb)

            prev = None
            for o in range(OS):
                w_t = []
                for ic in range(IC):
                    w = wp.tile([128, D], MMDT, tag=f"w{ic}")
                    nc.sync.dma_start(out=w, in_=w1s_d.ap()[o, ic, :, :])
                    w_t.append(w)
                for bp in range(NBP):
                    # stage1: M[j, x2] for this (o, bp)
                    m2 = mp.tile([128, IC, 2, L], MMDT, tag="m2")
                    for jt in range(IC):
                        p1 = ps1.tile([128, 2 * L], F32, tag="p1")
                        for ic in range(IC):
                            nc.tensor.matmul(
                                p1,
                                w_t[ic][:, jt * 128:(jt + 1) * 128],
                                xts[bp][:, ic, :, :],
                                start=(ic == 0), stop=(ic == IC - 1))
                        nc.vector.tensor_scalar_add(
                            m2[:, jt, :, :], p1, w2b_sb[:, jt, o:o + 1])
                    if prev is not None:
                        stage2(*prev)
                    prev = (o, bp, m2)
            stage2(*prev)

    nc.compile()
    _CACHE[mmdt_name] = nc
    return nc


def make_in_maps(input1, input2, w1, w2):
    """Host-side data marshaling (sharding + layout)."""
    input1 = np.asarray(input1, dtype=np.float32)
    input2 = np.asarray(input2, dtype=np.float32)
    w1 = np.asarray(w1, dtype=np.float32)
    w2 = np.asarray(w2, dtype=np.float32)

    x1t = np.ascontiguousarray(input1.transpose(0, 2, 1))      # [B, D, L]
    x2t = np.ascontiguousarray(input2.transpose(0, 2, 1))      # [B, D, L]
    ones = np.ones((1, 128), dtype=np.float32)

    in_maps = []
    for c in range(NCORES):
        sl = slice(c * OS, (c + 1) * OS)
        w1s = np.ascontiguousarray(
            w1[:, sl, :].transpose(1, 0, 2)).reshape(OS, IC, 128, D)
        # SBUF layout is [i_in_chunk(128 partitions), chunk, o]
        w2a = np.ascontiguousarray(
            w2[:D, sl].reshape(IC, 128, OS).transpose(1, 0, 2))
        w2b = np.ascontiguousarray(
            w2[D:2 * D, sl].reshape(IC, 128, OS).transpose(1, 0, 2))
        bias = np.ascontiguousarray(w2[2 * D:2 * D + 1, sl])
        in_maps.append({"x1t": x1t, "x2t": x2t, "w1s": w1s,
                        "w2a": w2a, "w2b": w2b, "bias": bias, "ones": ones})
    return in_maps


def kernel(input1, input2, w1, w2):
    from concourse.bass_utils import run_bass_kernel_spmd

    in_maps = make_in_maps(input1, input2, w1, w2)
    nc = _build("float32r")
    res = run_bass_kernel_spmd(nc, in_maps, core_ids=list(range(NCORES)))

    out = np.empty((B, L, L, O), dtype=np.float32)
    for c in range(NCORES):
        # per-core result is [B, OS, L, L] -> [B, L, L, OS]
        out[:, :, :, c * OS:(c + 1) * OS] = \
            res.results[c]["out"].transpose(0, 2, 3, 1)
    return out


# revision 15
# speedup vs baseline: 253.5458x; 253.5458x over previous
"""Biaffine kernel for Trainium2 (8 NeuronCores, Bass/Tile).

out[b,x,y,o] = sum_ij X[b,x,i] w1[i,o,j] Y[b,y,j]
             + (X[b] @ w2[:D])[x,o] + (Y[b] @ w2[D:2D])[y,o] + w2[2D][o]

Sharding: tensor-parallel over o (the w1/w2 out_size axis): core c owns
o in [16c, 16c+16). Each core reads the full (transposed) inputs, its w1/w2
slice, and writes out[b, o_local, x, y]; the host reorders to [b,x,y,o].

Per-core schedule, per (o, batch-pair):
  stage1: M[j, x2] = sum_i W_o[i,j] XT[i, x2]        (PE, fp32r, N=512)
  fold:   M += w2b[j,o]      (DVE tensor_scalar_add on PSUM->SBUF copy)
  stage2: out[x, y] = sum_j M[j, x] YT[j, y]         (PE, fp32r, N=256)
  fold:   out += c1b[x, o]   (ACT Identity+bias on PSUM->SBUF copy)
where c1b = X[b] @ w2a + bias is computed once per (b, x-tile) upfront.
"""

import numpy as np

B, L, D, O = 8, 256, 512, 128
NCORES = 8
OS = O // NCORES     # 16 o-channels per core
IC = D // 128        # 4 contraction chunks of 128
NBP = B // 2         # 4 batch pairs (stage1 moving dim = 2*L = 512)

_CACHE = {}


def _build(mmdt_name: str, n_reps: int = 1, variant: str = "full"):
    """Build + compile the per-core Bass program (same program on all cores).

    n_reps > 1 repeats the main loop inside the NEFF (timing amplification
    for benchmarking only; results are identical since it rewrites the same
    outputs).
    """
    import concourse.tile as tile
    from concourse import bacc, mybir

    key = (mmdt_name, n_reps, variant)
    if key in _CACHE:
        return _CACHE[key]

    F32 = mybir.dt.float32
    MMDT = getattr(mybir.dt, mmdt_name)

    nc = bacc.Bacc("TRN2", target_bir_lowering=False, debug=False,
                   num_devices=NCORES)

    x1t_d = nc.dram_tensor("x1t", [B, D, L], MMDT, kind="ExternalInput")
    x2t_d = nc.dram_tensor("x2t", [B, D, L], MMDT, kind="ExternalInput")
    w1s_d = nc.dram_tensor("w1s", [OS, IC, 128, D], MMDT, kind="ExternalInput")
    w2a_d = nc.dram_tensor("w2a", [128, IC, OS], MMDT, kind="ExternalInput")
    w2b_d = nc.dram_tensor("w2b", [128, IC, OS], F32, kind="ExternalInput")
    bias_d = nc.dram_tensor("bias", [1, OS], MMDT, kind="ExternalInput")
    ones_d = nc.dram_tensor("ones", [1, 128], MMDT, kind="ExternalInput")
    out_d = nc.dram_tensor("out", [B, OS, L, L], F32, kind="ExternalOutput")


    if variant == "stub":
        # minimal program with identical I/O signature (absolute-timing baseline)
        with tile.TileContext(nc) as tc:
            with tc.tile_pool(name="sb", bufs=1) as sb:
                t = sb.tile([128, IC, OS], F32, tag="t")
                o_sb = sb.tile([128, L], F32, tag="o")
                nc.sync.dma_start(out=t, in_=w2b_d.ap())
                nc.vector.memset(o_sb, 0.0)
                nc.vector.tensor_scalar_add(o_sb[:, 0:OS * IC],
                                            o_sb[:, 0:OS * IC], t[:, 0, 0:1])
                nc.sync.dma_start(out=out_d.ap()[0, 0, 0:128, :], in_=o_sb)
        nc.compile()
        _CACHE[key] = nc
        return nc

    ps2_bufs = 6 if variant in ("v2", "o8", "v4", "v5", "v6") else 4
    with tile.TileContext(nc) as tc:
        with tc.tile_pool(name="small", bufs=1) as small, \
             tc.tile_pool(name="xy", bufs=1) as xy, \
             tc.tile_pool(name="wp", bufs=4) as wp, \
             tc.tile_pool(name="mp", bufs=3) as mp, \
             tc.tile_pool(name="op", bufs=8) as op, \
             tc.tile_pool(name="ps1", bufs=2, space="PSUM") as ps1:

            # --- small persistent tiles ---
            w2a_sb = small.tile([128, IC, OS], MMDT, tag="w2a")
            w2b_sb = small.tile([128, IC, OS], F32, tag="w2b")
            bias_sb = small.tile([1, OS], MMDT, tag="bias")
            ones_sb = small.tile([1, 128], MMDT, tag="ones")
            c1b_sb = small.tile([128, B * 2 * OS], F32, tag="c1b")
            nc.sync.dma_start(out=w2a_sb, in_=w2a_d.ap())
            nc.sync.dma_start(out=w2b_sb, in_=w2b_d.ap())
            nc.sync.dma_start(out=bias_sb, in_=bias_d.ap())
            nc.sync.dma_start(out=ones_sb, in_=ones_d.ap())

            # --- prefetch first W tiles so the o=0 stage1 isn't gated on the
            # (larger) input loads finishing first on the same DMA queue ---
            w_cache = {}
            # v5: W loads ride the Activation HWDGE ring so they never queue
            # behind the (in-order) sync ring's input loads + output stores
            w_eng = nc.scalar if variant == "v5" else nc.sync
            if variant in ("v2", "o8", "v3", "v4", "v5", "v6"):
                for o in range(2):
                    w_t = []
                    for ic in range(IC):
                        w = wp.tile([128, D], MMDT, tag=f"w{ic}")
                        w_eng.dma_start(out=w, in_=w1s_d.ap()[o, ic, :, :])
                        w_t.append(w)
                    w_cache[o] = w_t

            # --- transposed inputs, all batches resident: [i%128, ic, b_in, x] ---
            xts, yts = [], []
            for bp in range(NBP):
                xt = xy.tile([128, IC, 2, L], MMDT, tag=f"xt{bp}")
                yt = xy.tile([128, IC, 2, L], MMDT, tag=f"yt{bp}")
                for b_in in range(2):
                    b = 2 * bp + b_in
                    for ic in range(IC):
                        in_eng = nc.scalar if (variant == "v5" and ic % 2) \
                            else nc.sync
                        in_eng.dma_start(
                            out=xt[:, ic, b_in, :],
                            in_=x1t_d.ap()[b, ic * 128:(ic + 1) * 128, :])
                        in_eng.dma_start(
                            out=yt[:, ic, b_in, :],
                            in_=x2t_d.ap()[b, ic * 128:(ic + 1) * 128, :])
                xts.append(xt)
                yts.append(yt)

            # --- c1b[x, (b,xt,o)] = X[b] @ w2a + bias ---
            psc_pool = [None]

            def emit_c1b(b):
                psc = psc_pool[0]
                bp, b_in = divmod(b, 2)
                for xt_i in range(2):
                    # v4 borrows stage2's psum slots (same tag) so no extra
                    # PSUM banks are reserved for this startup-only work
                    pc = psc.tile([128, OS], F32,
                                  tag="p2" if variant == "v4" else "pc")
                    for ic in range(IC):
                        nc.tensor.matmul(
                            pc,
                            xts[bp][:, ic, b_in, xt_i * 128:(xt_i + 1) * 128],
                            w2a_sb[:, ic, :],
                            start=(ic == 0), stop=False)
                    nc.tensor.matmul(
                        pc, ones_sb[0:1, :], bias_sb[0:1, :],
                        start=False, stop=True)
                    nc.vector.tensor_copy(
                        c1b_sb[:, (b * 2 + xt_i) * OS:(b * 2 + xt_i + 1) * OS],
                        pc)

            if variant not in ("v3", "v4"):
                with tc.tile_pool(name="psc", bufs=2, space="PSUM") as psc:
                    psc_pool[0] = psc
                    for b in range(B):
                        emit_c1b(b)

            if variant == "v3":
                # c1b psum shares the pool budget with ps2 (2 + 4 + 2 = 8 banks)
                psc_ctx = tc.tile_pool(name="psc", bufs=2, space="PSUM")
                psc_pool[0] = psc_ctx.__enter__()
                ps2_bufs = 4
            ps2_ctx = tc.tile_pool(name="ps2", bufs=ps2_bufs, space="PSUM")
            ps2 = ps2_ctx.__enter__()
            if variant == "v4":
                psc_pool[0] = ps2

            # --- main loop: software-pipelined over (o, bp) ---
            def stage2(o, bp, m2):
                for b_in in range(2):
                    b = 2 * bp + b_in
                    for xt_i in range(2):
                        p2 = ps2.tile([128, L], F32, tag="p2")
                        for jc in range(IC):
                            nc.tensor.matmul(
                                p2,
                                m2[:, jc, b_in, xt_i * 128:(xt_i + 1) * 128],
                                yts[bp][:, jc, b_in, :],
                                start=(jc == 0), stop=(jc == IC - 1))
                        o_sb = op.tile([128, L], F32, tag="osb")
                        c1col = c1b_sb[:, (b * 2 + xt_i) * OS + o:
                                       (b * 2 + xt_i) * OS + o + 1]
                        if variant == "dvecopy":
                            nc.vector.tensor_scalar_add(o_sb, p2, c1col)
                        elif variant == "v6" and xt_i == 1:
                            # balance stage2 psum drains across DVE and ACT so
                            # neither engine gates ps2 slot reuse
                            nc.vector.tensor_scalar_add(o_sb, p2, c1col)
                        else:
                            nc.scalar.add(o_sb, p2, c1col)
                        if variant != "nodma" or (o == OS - 1 and bp == NBP - 1):
                            nc.sync.dma_start(
                                out=out_d.ap()[b, o, xt_i * 128:(xt_i + 1) * 128, :],
                                in_=o_sb)

            def stage1(o, bp, w_t):
                m2 = mp.tile([128, IC, 2, L], MMDT, tag="m2")
                for jt in range(IC):
                    p1 = ps1.tile([128, 2 * L], F32, tag="p1")
                    for ic in range(IC):
                        nc.tensor.matmul(
                            p1,
                            w_t[ic][:, jt * 128:(jt + 1) * 128],
                            xts[bp][:, ic, :, :],
                            start=(ic == 0), stop=(ic == IC - 1))
                    nc.vector.tensor_scalar_add(
                        m2[:, jt, :, :], p1, w2b_sb[:, jt, o:o + 1])
                return m2

            os_eff = OS // 2 if variant == "o8" else OS

            def emit_main():
                # software-pipelined: stage2 for (o,bp) runs one step behind
                # stage1 so the PE never waits on the DVE M-copies.
                prev = None
                for o in range(os_eff):
                    if o in w_cache:
                        w_t = w_cache.pop(o)
                    else:
                        w_t = []
                        for ic in range(IC):
                            w = wp.tile([128, D], MMDT, tag=f"w{ic}")
                            w_eng.dma_start(out=w, in_=w1s_d.ap()[o, ic, :, :])
                            w_t.append(w)
                    for bp in range(NBP):
                        m2 = stage1(o, bp, w_t)
                        if variant in ("v3", "v4") and o == 0:
                            emit_c1b(2 * bp)
                            emit_c1b(2 * bp + 1)
                        if prev is not None:
                            stage2(*prev)
                        prev = (o, bp, m2)
                stage2(*prev)

            if n_reps == 1:
                emit_main()
            else:
                with tc.For_i(0, n_reps, 1):
                    emit_main()
            ps2_ctx.__exit__(None, None, None)
            if variant == "v3":
                psc_ctx.__exit__(None, None, None)

    nc.compile()
    _CACHE[key] = nc
    return nc


def make_in_maps(input1, input2, w1, w2):
    """Host-side data marshaling (sharding + layout)."""
    input1 = np.asarray(input1, dtype=np.float32)
    input2 = np.asarray(input2, dtype=np.float32)
    w1 = np.asarray(w1, dtype=np.float32)
    w2 = np.asarray(w2, dtype=np.float32)

    x1t = np.ascontiguousarray(input1.transpose(0, 2, 1))      # [B, D, L]
    x2t = np.ascontiguousarray(input2.transpose(0, 2, 1))      # [B, D, L]
    ones = np.ones((1, 128), dtype=np.float32)

    in_maps = []
    for c in range(NCORES):
        sl = slice(c * OS, (c + 1) * OS)
        w1s = np.ascontiguousarray(
            w1[:, sl, :].transpose(1, 0, 2)).reshape(OS, IC, 128, D)
        # SBUF layout is [i_in_chunk(128 partitions), chunk, o]
        w2a = np.ascontiguousarray(
            w2[:D, sl].reshape(IC, 128, OS).transpose(1, 0, 2))
        w2b = np.ascontiguousarray(
            w2[D:2 * D, sl].reshape(IC, 128, OS).transpose(1, 0, 2))
        bias = np.ascontiguousarray(w2[2 * D:2 * D + 1, sl])
        in_maps.append({"x1t": x1t, "x2t": x2t, "w1s": w1s,
                        "w2a": w2a, "w2b": w2b, "bias": bias, "ones": ones})
    return in_maps


def kernel(input1, input2, w1, w2):
    from concourse.bass_utils import run_bass_kernel_spmd

    in_maps = make_in_maps(input1, input2, w1, w2)
    nc = _build("float32r", 1, "v2")
    res = run_bass_kernel_spmd(nc, in_maps, core_ids=list(range(NCORES)))

    out = np.empty((B, L, L, O), dtype=np.float32)
    for c in range(NCORES):
        # per-core result is [B, OS, L, L] -> [B, L, L, OS]
        out[:, :, :, c * OS:(c + 1) * OS] = \
            res.results[c]["out"].transpose(0, 2, 3, 1)
    return out

